# revision 38
# baseline (speedup 1.0000x reference)
"""Trainium2 Bass kernel for the attention-scoring module:

    energy   = enc @ W.T + b           # [B,S,H]
    scores   = einsum('bh,bsh->bs', hidden, energy)
    out      = softmax(scores, axis=-1)[:, None, :]

Algebraic fusion: scores[b,s] = (hidden[b] @ W) . enc[b,s] + hidden[b].b,
and the bias term is constant per row so it cancels in the softmax.  The
kernel therefore only streams enc once (memory bound), computing
v[b] = hidden[b] @ W on-device first (per-partition scale + PE
ones-matmul partition reduction; an 8-core ReduceScatter was tried and
measured at ~76us under this runtime's ncfw ring, so W is replicated).

Streaming (all rates HW-measured on this runtime):
- One HWDGE ring sustains ~340 GB/s; the scalar ring adds ~75-160 (it
  is bistable run-to-run, so it carries only 6.5 MB of late-deadline
  tiles; everything critical rides the sync ring).
- Engines round-robin between rings per DESCRIPTOR, so every early
  transfer uses >=8KB-contiguous per-partition descriptors; hidden/ones
  (64-512B descriptors) are packed into the first W quarter instead of
  getting their own DMAs.
- W is host-preshuffled to Wr[p,c,h] = W[c*128+p,h] and sent as four
  1MB quarters so the v chain pipelines with their arrival; the PE is
  HAM-warmed with dummy matmuls on the first quarter's data.  The
  quarters borrow enc6-pool slots, freed at ~35us right before the
  first enc slot-waits would begin (effective bufs=6, no DMA
  slot-block stalls).

enc uses an s = 32*p + t layout, so the scores tile [128 part, 32 col]
lands in exact HBM output order - no output transpose.  The first b0
tile rides the otherwise-idle scalar ring head (arrives ~15us, when v
is ready); the final b1 tiles are 512KB so the post-stream DVE backlog
is a single 1.3us op.

Softmax shift: the global max over score columns 0..30 (computed on
DVE/GpSimd/ACT while the last column's tile is in flight) is the exp
shift for all 32 columns.  Softmax is shift-invariant, so this is exact
unless col 31's max exceeds the shift by >85 - impossible for
randn-scale scores (typical gap <10, exp overflows only at 88).

Sharding: data-parallel over batch; 16 batches / 8 cores = 2 per core.

Self-contained: hardcodes all shapes; only imports concourse/numpy.
"""

import numpy as np

B, S, H = 16, 4096, 1024
NCORES = 8
BPC = B // NCORES   # batches per core = 2
P = 128             # partitions
HC = H // P         # 8 contraction chunks for v = hidden @ W
NCOL = S // P       # 32 score columns per batch (s = p*NCOL + t)
SPLIT0 = [2, 2, 4, 6, 6, 6, 6]
SPLIT1 = [6, 6, 6, 6, 4, 2, 1, 1]
# ring assignment: tile index within the concatenated (b0|b1) tile list
A_RING = {0, 9, 11, 13}     # D0, E2, E4, E6 -> scalar ring (6.5MB:
# the scalar ring's rate is bistable run-to-run; this load meets every
# amr deadline with >10us margin even in its slow mode)
WQEXT = HC * BPC + P        # hTr + ones packed behind W quarter 0

_PROGRAM = None


def _build_program():
    import concourse.bacc as bacc
    import concourse.bass_isa as bass_isa
    import concourse.mybir as mybir
    import concourse.tile as tile

    f32 = mybir.dt.float32
    nc = bacc.Bacc("TRN2", target_bir_lowering=False, debug=False)

    enc_d = nc.dram_tensor("enc", [BPC, S, H], f32, kind="ExternalInput").ap()
    wq_d = [
        nc.dram_tensor(
            "wq0", [P, 2 * H + WQEXT], f32, kind="ExternalInput"
        ).ap()
    ] + [
        nc.dram_tensor(f"wq{i}", [P, 2 * H], f32, kind="ExternalInput").ap()
        for i in range(1, 4)
    ]
    out_d = nc.dram_tensor("out", [BPC, S], f32, kind="ExternalOutput").ap()

    with tile.TileContext(nc) as tc:
        with (
            tc.tile_pool(name="singles", bufs=1) as singles,
            tc.tile_pool(name="enc6", bufs=6) as enc6,
            tc.tile_pool(name="enc4", bufs=1) as enc4,
            tc.tile_pool(name="enc2", bufs=2) as enc2,
            tc.tile_pool(name="enc1", bufs=2) as enc1,
            tc.tile_pool(name="smallp", bufs=2) as smallp,
            tc.tile_pool(name="prodp", bufs=2) as prodp,
            tc.tile_pool(name="vps", bufs=2, space="PSUM") as vps,
            tc.tile_pool(name="warmp", bufs=1, space="PSUM") as warmp,
        ):
            # ---- W quarters, pipelined at the sync ring head
            # W quarters live in the enc6 pool: they free their slots
            # at ~35us (after the last prod), exactly when the first enc
            # tile slot-waits would begin, so the pool effectively has 6
            # buffers for the stream - removing the DMA slot-block stall
            # mode and giving scalar-ring tiles earlier gates.
            wq_sb = [
                enc6.tile(
                    [P, 2 * H + (WQEXT if i == 0 else 0)],
                    f32,
                    name=f"wq{i}",
                    tag="et",
                )
                for i in range(4)
            ]
            # all W quarters head the sync ring.  The pre-v0 phase is
            # bounded by a ~190 GB/s global early-DMA budget (measured:
            # identical v0 readiness with W split across sync/scalar/
            # gpsimd rings in any combination), so placement only
            # affects robustness - and sync is the least-noisy ring.
            for i in range(4):
                nc.sync.dma_start(out=wq_sb[i], in_=wq_d[i])
            hTr_sb = wq_sb[0][:, 2 * H:2 * H + HC * BPC]
            ones_sb = wq_sb[0][:, 2 * H + HC * BPC:2 * H + WQEXT]

            def wchunk(c):
                return wq_sb[c // 2][:, (c % 2) * H:(c % 2 + 1) * H]

            # enc tiles, s = 32*p + t: a tile covering t0..t0+T gives
            # each partition a T*4KB contiguous HBM read.
            pools = {6: enc6, 4: enc4, 2: enc2, 1: enc1}
            enc_tiles = {}  # (b, t0) -> tile
            enc_view = [
                enc_d[b].rearrange("(p t) h -> p t h", t=NCOL) for b in range(BPC)
            ]
            tlist = []  # (qi, b, t0, T) in consumption order
            qi = 0
            for b, split in ((0, SPLIT0), (1, SPLIT1)):
                t0 = 0
                for T in split:
                    tlist.append((qi, b, t0, T))
                    qi += 1
                    t0 += T
            # tiles allocated in consumption order (pool slots rotate
            # in .tile() call order; an E2-early reorder measured 142us
            # twice vs 129-130 for this order - possibly ring-phase
            # noise, but unproven, so consumption order ships)
            alloc_order = list(range(len(tlist)))
            for i in alloc_order:
                _, b, t0, T = tlist[i]
                enc_tiles[(b, t0)] = pools[T].tile(
                    [P, T, H], f32, name=f"et{b}_{t0}", tag="et"
                )
            for qi, b, t0, T in tlist:
                eng = nc.scalar if qi in A_RING else nc.sync
                eng.dma_start(
                    out=enc_tiles[(b, t0)], in_=enc_view[b][:, t0:t0 + T, :]
                )

            # ---- PE HAM warm-up on W quarter 0 (lands ~13us) so the
            # fp32 v-chain matmuls run at 2.4GHz instead of cold 1.2.
            junk = singles.tile([P, H], f32)  # amr product dump (SBUF:
            # a PSUM dump was measured +65ns/op slower from DVE access cost)
            warm_ps = warmp.tile([P, 512], f32)
            for _ in range(5):
                nc.tensor.matmul(
                    warm_ps,
                    wq_sb[0][:, 0:128],
                    wq_sb[0][:, 512:1024],
                    start=True,
                    stop=True,
                )

            # ---- v[b] = hidden[b] @ W, replicated on all partitions:
            # prod[g,h] = W[g,h] * hidden[b,g] (ACT per-partition scale;
            # the DVE stays a pure amr stream), ones.T @ prod sums over
            # g on the PE -> [128, H] PSUM, then ACT copies to SBUF
            # (amr reads SBUF ~220ns/op faster than PSUM).
            v_sb = [
                singles.tile([P, H], f32, name=f"v_sb{b}") for b in range(BPC)
            ]
            for b in range(BPC):
                vp = vps.tile([P, H], f32, tag="v_ps", name=f"v_ps{b}")
                chunk_order = [0, 1, 2, 3, 4, 5, 6, 7]  # arrival order
                for ci, c in enumerate(chunk_order):
                    prod = prodp.tile([P, H], f32)
                    scl = hTr_sb[:, c * BPC + b:c * BPC + b + 1]
                    # all prods on ACT: keeps the DVE a pure amr stream
                    # (ACT pipelines them with the W quarter arrivals)
                    nc.scalar.mul(out=prod, in_=wchunk(c), mul=scl)
                    for hh in range(2):
                        nc.tensor.matmul(
                            vp[:, hh * 512:(hh + 1) * 512],
                            ones_sb,
                            prod[:, hh * 512:(hh + 1) * 512],
                            start=(ci == 0),
                            stop=(ci == HC - 1),
                        )
                # ACT copy emitted before batch 1's ACT prods so v0 is in
                # SBUF the moment its PSUM accumulation finishes
                nc.scalar.copy(v_sb[b], vp)

            # ---- stream: fused dot on DVE, softmax on ACT/GpSimd ----
            scores_t = [
                singles.tile([P, NCOL], f32, name=f"scores{b}") for b in range(BPC)
            ]
            sm = {}

            def amr_col(b, t0, tloc):
                col = t0 + tloc
                nc.vector.affine_mul_reduce(
                    out=junk,
                    accum_out=scores_t[b][:, col:col + 1],
                    in0=enc_tiles[(b, t0)][:, tloc, :],
                    in1=v_sb[b],
                    scale=1.0,
                    bias=0.0,
                )

            def early_max(b):
                # global max over cols 0..30; runs while col 31 is in flight
                rmax = smallp.tile([P, 1], f32, name=f"rmax{b}")
                nc.vector.tensor_reduce(
                    out=rmax, in_=scores_t[b][:, 0:NCOL - 1],
                    axis=mybir.AxisListType.X, op=mybir.AluOpType.max,
                )
                gmax = smallp.tile([P, 1], f32, name=f"gmax{b}")
                nc.gpsimd.partition_all_reduce(
                    gmax, rmax, channels=P, reduce_op=bass_isa.ReduceOp.max
                )
                negm = smallp.tile([P, 1], f32, name=f"negm{b}")
                nc.scalar.mul(out=negm, in_=gmax, mul=-1.0)
                sm[b] = {"negm": negm}

            def softmax_head(b):
                probs = smallp.tile([P, NCOL], f32, name=f"probs{b}")
                sume = smallp.tile([P, 1], f32, name=f"sume{b}")
                nc.scalar.activation(
                    out=probs,
                    in_=scores_t[b],
                    func=mybir.ActivationFunctionType.Exp,
                    bias=sm[b]["negm"],
                    scale=1.0,
                    accum_out=sume,
                )
                gsum = smallp.tile([P, 1], f32, name=f"gsum{b}")
                nc.gpsimd.partition_all_reduce(
                    gsum, sume, channels=P, reduce_op=bass_isa.ReduceOp.add
                )
                sm[b].update(probs=probs, gsum=gsum)

            def softmax_tail(b):
                rinv = smallp.tile([P, 1], f32, name=f"rinv{b}")
                nc.vector.reciprocal(rinv, sm[b]["gsum"])  # DVE
                pn = smallp.tile([P, NCOL], f32, name=f"pn{b}")
                # normalize on the DVE too: same engine as rinv (no
                # cross-engine hop) and fp32 tensor_scalar runs 2x mode
                nc.vector.tensor_scalar_mul(
                    out=pn, in0=sm[b]["probs"], scalar1=rinv
                )
                # scores layout [p, t] is exactly HBM order s = 32p + t;
                # outs ride the HWDGE ring tails (their 128B descriptors
                # would steal round-robin turns on a busy ring).
                eng = nc.scalar if b == 0 else nc.sync
                eng.dma_start(
                    out=out_d[b].rearrange("(p t) -> p t", t=NCOL), in_=pn
                )

            def cols_of(split):
                cols, t0 = [], 0
                for T in split:
                    cols += [(t0, tl) for tl in range(T)]
                    t0 += T
                return cols

            b0c, b1c = cols_of(SPLIT0), cols_of(SPLIT1)
            for t0, tl in b0c[:-1]:
                amr_col(0, t0, tl)
            early_max(0)
            amr_col(0, *b0c[-1])
            softmax_head(0)
            # batch 0's DVE rinv is emitted after two b1 amrs so the DVE
            # in-order stream never stalls waiting on b0's ACT/gpsimd chain
            amr_col(1, *b1c[0])
            amr_col(1, *b1c[1])
            softmax_tail(0)
            for t0, tl in b1c[2:-1]:
                amr_col(1, t0, tl)
            early_max(1)
            amr_col(1, *b1c[-1])
            softmax_head(1)
            softmax_tail(1)

    nc.compile()
    return nc


def _get_program():
    global _PROGRAM
    if _PROGRAM is None:
        _PROGRAM = _build_program()
    return _PROGRAM


def make_in_maps(hidden, encoder_outputs, W):
    hidden = np.asarray(hidden, dtype=np.float32)
    encoder_outputs = np.asarray(encoder_outputs, dtype=np.float32)
    W = np.asarray(W, dtype=np.float32)
    # Wr[p, c, h] = W[c*128+p, h]: per-partition-contiguous descriptors
    Wr = np.ascontiguousarray(W.reshape(HC, P, H).transpose(1, 0, 2))
    wq = [np.ascontiguousarray(Wr[:, 2 * i:2 * i + 2].reshape(P, 2 * H))
          for i in range(4)]
    in_maps = []
    for r in range(NCORES):
        sl = slice(BPC * r, BPC * (r + 1))
        hshard = hidden[sl]  # [BPC, H]
        # hTr[p, c*BPC+b] = hidden[b, c*128+p]
        hTr = hshard.reshape(BPC, HC, P).transpose(2, 1, 0).reshape(P, HC * BPC)
        wq0 = np.ascontiguousarray(np.concatenate(
            [wq[0], hTr, np.ones((P, P), dtype=np.float32)], axis=1
        ))
        in_maps.append({
            "enc": np.ascontiguousarray(encoder_outputs[sl]),
            "wq0": wq0,
            "wq1": wq[1],
            "wq2": wq[2],
            "wq3": wq[3],
        })
    return in_maps


def kernel(hidden, encoder_outputs, W, b):
    """Full-input entry point. `b` provably cancels in the softmax (it only
    adds a per-row constant to the scores) and is unused."""
    from concourse.bass_utils import run_bass_kernel_spmd

    nc = _get_program()
    in_maps = make_in_maps(hidden, encoder_outputs, W)
    res = run_bass_kernel_spmd(nc, in_maps, core_ids=list(range(NCORES)))
    out = np.concatenate([r["out"] for r in res.results], axis=0)  # [16, 4096]
    return out.reshape(B, 1, S).astype(np.float32)


# revision 39
# speedup vs baseline: 1.1195x; 1.1195x over previous
"""Trainium2 Bass kernel for the attention-scoring module:

    energy   = enc @ W.T + b           # [B,S,H]
    scores   = einsum('bh,bsh->bs', hidden, energy)
    out      = softmax(scores, axis=-1)[:, None, :]

Algebraic fusion: scores[b,s] = (hidden[b] @ W) . enc[b,s] + hidden[b].b,
and the bias term is constant per row so it cancels in the softmax.  The
kernel therefore only streams enc once (memory bound), computing
v[b] = hidden[b] @ W on-device first (per-partition scale + PE
ones-matmul partition reduction; an 8-core ReduceScatter was tried and
measured at ~76us under this runtime's ncfw ring, so W is replicated).

Streaming (all rates HW-measured on this runtime):
- One HWDGE ring sustains ~340 GB/s; the scalar ring adds ~75-160 (it
  is bistable run-to-run, so it carries only 6.5 MB of late-deadline
  tiles; everything critical rides the sync ring).
- Engines round-robin between rings per DESCRIPTOR, so every early
  transfer uses >=8KB-contiguous per-partition descriptors; hidden/ones
  (64-512B descriptors) are packed into the first W quarter instead of
  getting their own DMAs.
- W is host-preshuffled to Wr[p,c,h] = W[c*128+p,h] and sent as four
  1MB quarters so the v chain pipelines with their arrival; the PE is
  HAM-warmed with dummy matmuls on the first quarter's data.  The
  quarters borrow enc6-pool slots, freed at ~35us right before the
  first enc slot-waits would begin (effective bufs=6, no DMA
  slot-block stalls).

enc uses an s = 32*p + t layout, so the scores tile [128 part, 32 col]
lands in exact HBM output order - no output transpose.  The first b0
tile rides the otherwise-idle scalar ring head (arrives ~15us, when v
is ready); the final b1 tiles are 512KB so the post-stream DVE backlog
is a single 1.3us op.

Softmax shift: the global max over score columns 0..30 (computed on
DVE/GpSimd/ACT while the last column's tile is in flight) is the exp
shift for all 32 columns.  Softmax is shift-invariant, so this is exact
unless col 31's max exceeds the shift by >85 - impossible for
randn-scale scores (typical gap <10, exp overflows only at 88).

Sharding: data-parallel over batch; 16 batches / 8 cores = 2 per core.

Self-contained: hardcodes all shapes; only imports concourse/numpy.
"""

import numpy as np

B, S, H = 16, 4096, 1024
NCORES = 8
BPC = B // NCORES   # batches per core = 2
P = 128             # partitions
HC = H // P         # 8 contraction chunks for v = hidden @ W
NCOL = S // P       # 32 score columns per batch (s = p*NCOL + t)
SPLIT0 = [2, 2, 4, 6, 6, 6, 6]
SPLIT1 = [6, 6, 6, 6, 4, 2, 1, 1]
# ring assignment: tile index within the concatenated (b0|b1) tile list
A_RING = {0, 9, 11, 13}     # D0, E2, E4, E6 -> scalar ring (6.5MB:
# the scalar ring's rate is bistable run-to-run; this load meets every
# amr deadline with >10us margin even in its slow mode)
WQEXT = HC * BPC + P        # hTr + ones packed behind W quarter 0

_PROGRAM = None


def _build_program():
    import concourse.bacc as bacc
    import concourse.bass_isa as bass_isa
    import concourse.mybir as mybir
    import concourse.tile as tile

    f32 = mybir.dt.float32
    nc = bacc.Bacc("TRN2", target_bir_lowering=False, debug=False)

    enc_d = nc.dram_tensor("enc", [BPC, S, H], f32, kind="ExternalInput").ap()
    wq_d = [
        nc.dram_tensor(
            "wq0", [P, 2 * H + WQEXT], f32, kind="ExternalInput"
        ).ap()
    ] + [
        nc.dram_tensor(f"wq{i}", [P, 2 * H], f32, kind="ExternalInput").ap()
        for i in range(1, 4)
    ]
    out_d = nc.dram_tensor("out", [BPC, S], f32, kind="ExternalOutput").ap()

    with tile.TileContext(nc) as tc:
        with (
            tc.tile_pool(name="singles", bufs=1) as singles,
            tc.tile_pool(name="enc6", bufs=6) as enc6,
            tc.tile_pool(name="enc4", bufs=1) as enc4,
            tc.tile_pool(name="enc2", bufs=2) as enc2,
            tc.tile_pool(name="enc1", bufs=2) as enc1,
            tc.tile_pool(name="smallp", bufs=2) as smallp,
            tc.tile_pool(name="prodp", bufs=2) as prodp,
            tc.tile_pool(name="vps", bufs=2, space="PSUM") as vps,
            tc.tile_pool(name="warmp", bufs=1, space="PSUM") as warmp,
        ):
            # ---- W quarters, pipelined at the sync ring head
            # W quarters live in the enc6 pool: they free their slots
            # at ~35us (after the last prod), exactly when the first enc
            # tile slot-waits would begin, so the pool effectively has 6
            # buffers for the stream - removing the DMA slot-block stall
            # mode and giving scalar-ring tiles earlier gates.
            wq_sb = [
                enc6.tile(
                    [P, 2 * H + (WQEXT if i == 0 else 0)],
                    f32,
                    name=f"wq{i}",
                    tag="et",
                )
                for i in range(4)
            ]
            # all W quarters head the sync ring.  The pre-v0 phase is
            # bounded by a ~190 GB/s global early-DMA budget (measured:
            # identical v0 readiness with W split across sync/scalar/
            # gpsimd rings in any combination), so placement only
            # affects robustness - and sync is the least-noisy ring.
            for i in range(4):
                nc.sync.dma_start(out=wq_sb[i], in_=wq_d[i])
            hTr_sb = wq_sb[0][:, 2 * H:2 * H + HC * BPC]
            ones_sb = wq_sb[0][:, 2 * H + HC * BPC:2 * H + WQEXT]

            def wchunk(c):
                return wq_sb[c // 2][:, (c % 2) * H:(c % 2 + 1) * H]

            # enc tiles, s = 32*p + t: a tile covering t0..t0+T gives
            # each partition a T*4KB contiguous HBM read.
            pools = {6: enc6, 4: enc4, 2: enc2, 1: enc1}
            enc_tiles = {}  # (b, t0) -> tile
            enc_view = [
                enc_d[b].rearrange("(p t) h -> p t h", t=NCOL) for b in range(BPC)
            ]
            tlist = []  # (qi, b, t0, T) in consumption order
            qi = 0
            for b, split in ((0, SPLIT0), (1, SPLIT1)):
                t0 = 0
                for T in split:
                    tlist.append((qi, b, t0, T))
                    qi += 1
                    t0 += T
            # tiles allocated in consumption order (pool slots rotate
            # in .tile() call order; an E2-early reorder measured 142us
            # twice vs 129-130 for this order - possibly ring-phase
            # noise, but unproven, so consumption order ships)
            alloc_order = list(range(len(tlist)))
            for i in alloc_order:
                _, b, t0, T = tlist[i]
                enc_tiles[(b, t0)] = pools[T].tile(
                    [P, T, H], f32, name=f"et{b}_{t0}", tag="et"
                )
            for qi, b, t0, T in tlist:
                eng = nc.scalar if qi in A_RING else nc.sync
                eng.dma_start(
                    out=enc_tiles[(b, t0)], in_=enc_view[b][:, t0:t0 + T, :]
                )

            # ---- PE HAM warm-up on W quarter 0 (lands ~13us) so the
            # fp32 v-chain matmuls run at 2.4GHz instead of cold 1.2.
            junk = singles.tile([P, H], f32)  # amr product dump (SBUF:
            # a PSUM dump was measured +65ns/op slower from DVE access cost)
            warm_ps = warmp.tile([P, 512], f32)
            for _ in range(5):
                nc.tensor.matmul(
                    warm_ps,
                    wq_sb[0][:, 0:128],
                    wq_sb[0][:, 512:1024],
                    start=True,
                    stop=True,
                )

            # ---- v[b] = hidden[b] @ W, replicated on all partitions:
            # prod[g,h] = W[g,h] * hidden[b,g] (ACT per-partition scale;
            # the DVE stays a pure amr stream), ones.T @ prod sums over
            # g on the PE -> [128, H] PSUM, then ACT copies to SBUF
            # (amr reads SBUF ~220ns/op faster than PSUM).
            v_sb = [
                singles.tile([P, H], f32, name=f"v_sb{b}") for b in range(BPC)
            ]
            for b in range(BPC):
                vp = vps.tile([P, H], f32, tag="v_ps", name=f"v_ps{b}")
                chunk_order = [0, 1, 2, 3, 4, 5, 6, 7]  # arrival order
                for ci, c in enumerate(chunk_order):
                    prod = prodp.tile([P, H], f32)
                    scl = hTr_sb[:, c * BPC + b:c * BPC + b + 1]
                    # all prods on ACT: keeps the DVE a pure amr stream
                    # (ACT pipelines them with the W quarter arrivals)
                    nc.scalar.mul(out=prod, in_=wchunk(c), mul=scl)
                    for hh in range(2):
                        nc.tensor.matmul(
                            vp[:, hh * 512:(hh + 1) * 512],
                            ones_sb,
                            prod[:, hh * 512:(hh + 1) * 512],
                            start=(ci == 0),
                            stop=(ci == HC - 1),
                        )
                # ACT copy emitted before batch 1's ACT prods so v0 is in
                # SBUF the moment its PSUM accumulation finishes
                nc.scalar.copy(v_sb[b], vp)

            # ---- stream: fused dot on DVE, softmax on ACT/GpSimd ----
            scores_t = [
                singles.tile([P, NCOL], f32, name=f"scores{b}") for b in range(BPC)
            ]
            sm = {}

            def amr_col(b, t0, tloc):
                col = t0 + tloc
                nc.vector.affine_mul_reduce(
                    out=junk,
                    accum_out=scores_t[b][:, col:col + 1],
                    in0=enc_tiles[(b, t0)][:, tloc, :],
                    in1=v_sb[b],
                    scale=1.0,
                    bias=0.0,
                )

            def early_max(b):
                # global max over cols 0..30; runs while col 31 is in flight
                rmax = smallp.tile([P, 1], f32, name=f"rmax{b}")
                nc.vector.tensor_reduce(
                    out=rmax, in_=scores_t[b][:, 0:NCOL - 1],
                    axis=mybir.AxisListType.X, op=mybir.AluOpType.max,
                )
                gmax = smallp.tile([P, 1], f32, name=f"gmax{b}")
                nc.gpsimd.partition_all_reduce(
                    gmax, rmax, channels=P, reduce_op=bass_isa.ReduceOp.max
                )
                negm = smallp.tile([P, 1], f32, name=f"negm{b}")
                nc.scalar.mul(out=negm, in_=gmax, mul=-1.0)
                sm[b] = {"negm": negm}

            def softmax_head(b):
                probs = smallp.tile([P, NCOL], f32, name=f"probs{b}")
                sume = smallp.tile([P, 1], f32, name=f"sume{b}")
                nc.scalar.activation(
                    out=probs,
                    in_=scores_t[b],
                    func=mybir.ActivationFunctionType.Exp,
                    bias=sm[b]["negm"],
                    scale=1.0,
                    accum_out=sume,
                )
                gsum = smallp.tile([P, 1], f32, name=f"gsum{b}")
                nc.gpsimd.partition_all_reduce(
                    gsum, sume, channels=P, reduce_op=bass_isa.ReduceOp.add
                )
                sm[b].update(probs=probs, gsum=gsum)

            def softmax_tail(b):
                rinv = smallp.tile([P, 1], f32, name=f"rinv{b}")
                nc.vector.reciprocal(rinv, sm[b]["gsum"])  # DVE
                pn = smallp.tile([P, NCOL], f32, name=f"pn{b}")
                # normalize on the DVE too: same engine as rinv (no
                # cross-engine hop) and fp32 tensor_scalar runs 2x mode
                nc.vector.tensor_scalar_mul(
                    out=pn, in0=sm[b]["probs"], scalar1=rinv
                )
                # scores layout [p, t] is exactly HBM order s = 32p + t;
                # outs ride the HWDGE ring tails (their 128B descriptors
                # would steal round-robin turns on a busy ring).
                eng = nc.scalar if b == 0 else nc.sync
                eng.dma_start(
                    out=out_d[b].rearrange("(p t) -> p t", t=NCOL), in_=pn
                )

            def cols_of(split):
                cols, t0 = [], 0
                for T in split:
                    cols += [(t0, tl) for tl in range(T)]
                    t0 += T
                return cols

            b0c = cols_of(SPLIT0)
            # batch 1 amrs in expected-arrival order: sync-ring tiles
            # (E0,E1,E3,E5) first, scalar-ring tiles (E2,E4) last, so a
            # slow scalar ring gets ~25us more delivery slack before its
            # columns are needed.  Softmax is order-invariant; cols 30/31
            # stay last so the early-max shift still covers 0..30.
            b1_sizes = {}
            t0 = 0
            for T in SPLIT1:
                b1_sizes[t0] = T
                t0 += T
            b1_tile_order = [0, 6, 18, 28, 12, 24, 30, 31]
            b1c = [
                (t0, tl) for t0 in b1_tile_order for tl in range(b1_sizes[t0])
            ]
            assert len(b1c) == NCOL and b1c[-1] == (31, 0) and b1c[-2] == (30, 0)
            for t0, tl in b0c[:-1]:
                amr_col(0, t0, tl)
            early_max(0)
            amr_col(0, *b0c[-1])
            softmax_head(0)
            # batch 0's DVE rinv is emitted after two b1 amrs so the DVE
            # in-order stream never stalls waiting on b0's ACT/gpsimd chain
            amr_col(1, *b1c[0])
            amr_col(1, *b1c[1])
            softmax_tail(0)
            for t0, tl in b1c[2:-1]:
                amr_col(1, t0, tl)
            early_max(1)
            amr_col(1, *b1c[-1])
            softmax_head(1)
            softmax_tail(1)

    nc.compile()
    return nc


def _get_program():
    global _PROGRAM
    if _PROGRAM is None:
        _PROGRAM = _build_program()
    return _PROGRAM


def make_in_maps(hidden, encoder_outputs, W):
    hidden = np.asarray(hidden, dtype=np.float32)
    encoder_outputs = np.asarray(encoder_outputs, dtype=np.float32)
    W = np.asarray(W, dtype=np.float32)
    # Wr[p, c, h] = W[c*128+p, h]: per-partition-contiguous descriptors
    Wr = np.ascontiguousarray(W.reshape(HC, P, H).transpose(1, 0, 2))
    wq = [np.ascontiguousarray(Wr[:, 2 * i:2 * i + 2].reshape(P, 2 * H))
          for i in range(4)]
    in_maps = []
    for r in range(NCORES):
        sl = slice(BPC * r, BPC * (r + 1))
        hshard = hidden[sl]  # [BPC, H]
        # hTr[p, c*BPC+b] = hidden[b, c*128+p]
        hTr = hshard.reshape(BPC, HC, P).transpose(2, 1, 0).reshape(P, HC * BPC)
        wq0 = np.ascontiguousarray(np.concatenate(
            [wq[0], hTr, np.ones((P, P), dtype=np.float32)], axis=1
        ))
        in_maps.append({
            "enc": np.ascontiguousarray(encoder_outputs[sl]),
            "wq0": wq0,
            "wq1": wq[1],
            "wq2": wq[2],
            "wq3": wq[3],
        })
    return in_maps


def kernel(hidden, encoder_outputs, W, b):
    """Full-input entry point. `b` provably cancels in the softmax (it only
    adds a per-row constant to the scores) and is unused."""
    from concourse.bass_utils import run_bass_kernel_spmd

    nc = _get_program()
    in_maps = make_in_maps(hidden, encoder_outputs, W)
    res = run_bass_kernel_spmd(nc, in_maps, core_ids=list(range(NCORES)))
    out = np.concatenate([r["out"] for r in res.results], axis=0)  # [16, 4096]
    return out.reshape(B, 1, S).astype(np.float32)
